# revision 58
# baseline (speedup 1.0000x reference)
"""CARAFE content-aware upsampling (scale=2, K=5, encoder 3x3) on 8 TRN2 NeuronCores.

Sharding: 8 shards = batch(4) x H-halves(2), pure data parallel (1-row halo
per shard handled host-side). Channel-major fp16 pipeline, fully pipelined at
(row-half x subgrid) granularity:

  1+2. fused conv           : the 1x1 compress conv is folded into the 3x3
                              encoder host-side (W_enc @ W_comp, exact since
                              zero-pad commutes through a bias-free 1x1);
                              9 accumulating PE matmuls per 8-row block read
                              the dx-shifted x copies directly
  3. e = exp(enc + b)       : ACT (comp_b folded into enc bias host-side)
  4. combined masses        : one PE matmul [100->68] = 36 shifted-tap masses
                              (dx-major order, partitions 0-35) + 4 softmax
                              denominators S (partitions 64-67)
  5. r = 1/S                : DVE reciprocal_approx_fast (via SBUF staging);
                              R9 one-hot PE matmul expands r to 36 rows;
                              DVE multiplies masses in place
  6. mask broadcast         : per (row-half, subgrid): bounce to DRAM on the
                              sync HWDGE ring, then a stride-0 SWDGE DMA
                              (16 SDMA engines, ~400 GB/s aggregate)
                              replicating [9,16,64] masses to all 128
                              partitions
  7. reassembly             : 3 DVE multiplies per chunk (dy-triples share an
                              overlapping-row window AP); 9 accumulating PE
                              matmuls per 512-px block sum the taps in PSUM
  8. out = fp16 staging     : ACT PSUM->SBUF interleaved copy; two 0.5MB
                              store DMAs per row-half split across the two
                              HWDGE rings; host casts fp16 -> fp32

All weights ride in a single [128, 1134] fp16 blob (one DMA, loaded first);
the fp32 encoder bias is bit-packed into two fp16 columns. Junk matmuls
(initial burst + gap fillers) hold the PE HAM clock gate at 8/8 (2.4 GHz)
through the mask phase and into reassembly; the Exp ACT table is preloaded
off the critical path.
"""

import numpy as np

SCALE, KK, EK = 2, 5, 3
B, C, H, W = 4, 128, 64, 64
CC, KC = 64, 100
HS = H // 2          # 32 interior rows per shard
PIX = HS * W
NCORES = 8

# blob column offsets (fp16 columns)
O_ID, O_WF, O_A, O_R9, O_EB = 0, 128, 1028, 1096, 1132
NCOL = 1134

_PROGRAM = None


def _build_A():
    """[100, 40] combine matrix: cols 0-35 = shifted-tap masses (dx-major
    within each subgrid), cols 36-39 = softmax denominators per subgrid."""
    A = np.zeros((KC, 40), dtype=np.float32)
    for r1 in range(2):
        for r2 in range(2):
            q = 2 * r1 + r2
            for i in range(KK):
                for j in range(KK):
                    dy = (r1 + i - 2) // 2
                    dx = (r2 + j - 2) // 2
                    tidx = (dx + 1) * 3 + (dy + 1)
                    A[4 * (5 * i + j) + q, q * 9 + tidx] += 1.0
            A[np.arange(q, KC, 4), 36 + q] = 1.0
    return A


def _build_program():
    import concourse.bass as bass
    import concourse.tile as tile
    from concourse.tile import add_dep_helper
    from concourse import bacc, mybir

    f32 = mybir.dt.float32
    f16 = mybir.dt.float16
    AF = mybir.ActivationFunctionType

    nc = bacc.Bacc("TRN2", target_bir_lowering=False, debug=False,
                   num_devices=NCORES)

    xin = nc.declare_dram_parameter("xs", [C, HS + 2, W], f32, isOutput=False)
    wbl = nc.declare_dram_parameter("wblob", [C, NCOL], f16, isOutput=False)
    out = nc.declare_dram_parameter("out", [C, 2 * HS, 2 * W], f16, isOutput=True)

    # masks bounced per row-half: [rh][q][tap][16][64] fp16
    mu_dram = nc.dram_tensor("mu_bounce", [2, 4, 9, 16, W], f16)

    with tile.TileContext(nc) as tc:
        with (
            tc.tile_pool(name="singles", bufs=1) as singles,
            tc.tile_pool(name="work", bufs=2) as work,
            tc.tile_pool(name="mc", bufs=3) as mc,
            tc.tile_pool(name="tp", bufs=3) as tp,
            tc.tile_pool(name="pse", bufs=2, space="PSUM") as pse,
            tc.tile_pool(name="psc", bufs=1, space="PSUM") as psc,
            tc.tile_pool(name="psr", bufs=1, space="PSUM") as psr,
            tc.tile_pool(name="psa", bufs=4, space="PSUM") as psa,
        ):
            # ---------------- persistent SBUF ----------------
            blob = singles.tile([C, NCOL], f16, tag="wblob")
            x16 = [singles.tile([C, HS + 2, W], f16, tag=f"x16_{d}",
                                name=f"x16_{d}")
                   for d in range(3)]  # dx = -1, 0, +1 pre-shifted copies
            e_sb = singles.tile([KC, HS, W], f16, tag="e_sb")
            mu16 = singles.tile([36, HS, W], f16, tag="mu16")
            out16 = [singles.tile([C, 16, 2, W, 2], f16, tag=f"o16_{rh}",
                                  name=f"o16_{rh}")
                     for rh in range(2)]

            id_sb = blob[:, O_ID : O_ID + 128]
            wf_t = [blob[:, O_WF + KC * t : O_WF + KC * (t + 1)]
                    for t in range(9)]
            A_sb = blob[0:KC, O_A : O_A + 68]
            R9_sb = blob[0:4, O_R9 : O_R9 + 36]
            eb_sb = blob[0:KC, O_EB : O_EB + 2].bitcast(f32)

            # ---------------- loads ----------------
            nc.sync.dma_start(out=blob, in_=wbl[:])
            # x load with fp32 -> fp16 cast (SWDGE), 4 splits for fast ramp
            for a, b in [(0, 9), (9, 17), (17, 25), (25, 34)]:
                nc.gpsimd.dma_start(out=x16[1][:, a:b, :], in_=xin[:, a:b, :])

            # PE warm-up: back-to-back junk matmuls while the x-load streams,
            # so the HAM clock gate reaches 8/8 (2.4 GHz) before the real
            # mask matmuls start instead of running them all at 1.2 GHz.
            warm = psa.tile([128, 8, W], f32, tag="acc", name="warmup")
            for _ in range(40):
                nc.tensor.matmul(warm[:, 0:2, :], id_sb, id_sb,
                                 start=True, stop=True, skip_group_check=True)

            # preload the Exp ACT table during the load phase so the first
            # real exp doesn't pay the ~1.3us table load on the mask chain
            tblw = work.tile([1, 1], f16, tag="tblw")
            nc.scalar.activation(tblw, blob[0:1, 0:1], AF.Exp)

            nc.vector.memset(x16[0][:, :, 0:1], 0.0)
            nc.vector.memset(x16[2][:, :, W - 1 : W], 0.0)
            # dx-shift copies split per x-load chunk so encoder block 0 can
            # start right after the first chunk lands instead of after the
            # whole x-load
            for a, b in [(0, 9), (9, 17), (17, 25), (25, 34)]:
                nc.vector.tensor_copy(x16[0][:, a:b, 1:W],
                                      x16[1][:, a:b, 0 : W - 1])
                nc.vector.tensor_copy(x16[2][:, a:b, 0 : W - 1],
                                      x16[1][:, a:b, 1:W])

            # ---------------- stages 2-5 for one 8-row block ----------------
            def emit_mask_block(blk):
                y0 = 8 * blk
                ps = pse.tile([KC, 8, W], f32, tag="pse", name=f"enc_{y0}")
                # fused compress+encoder conv: 9 accumulating matmuls read
                # the dx-shifted x copies directly (zeroed edge columns
                # implement the conv padding exactly)
                for di in range(3):
                    for dj in range(3):
                        t = di * 3 + dj
                        nc.tensor.matmul(ps, wf_t[t],
                                         x16[dj][:, y0 + di : y0 + di + 8, :],
                                         start=(t == 0), stop=(t == 8))
                nc.scalar.activation(e_sb[:, y0 : y0 + 8, :], ps, AF.Exp,
                                     bias=eb_sb, scale=1.0)
                # combine: 36 masses + 4 denominators in one matmul
                pc = psc.tile([68, 8, W], f32, tag="psc", name=f"cmb_{y0}")
                nc.tensor.matmul(pc, A_sb, e_sb[:, y0 : y0 + 8, :],
                                 start=True, stop=True)
                s32 = work.tile([4, 8, W], f32, tag="s32", name=f"s32_{y0}")
                r32 = work.tile([4, 8, W], f32, tag="r32", name=f"r32_{y0}")
                r16 = work.tile([4, 8, W], f16, tag="r16", name=f"r16_{y0}")
                m36 = work.tile([36, 8, W], f16, tag="m36", name=f"m36_{y0}")
                # the bitwise-seed reciprocal cannot read PSUM; stage via ACT
                # (a DVE copy here would head-of-line-block the products
                # behind row-half-1's PE combines)
                nc.scalar.copy(s32, pc[64:68])
                nc.vector.reciprocal_approx_fast(r32, s32)
                nc.vector.tensor_copy(r16, r32)
                nc.scalar.copy(m36, pc[0:36])
                pr = psr.tile([36, 8, W], f32, tag="psr", name=f"r36_{y0}")
                nc.tensor.matmul(pr, R9_sb, r16, start=True, stop=True)
                norm_ops[blk] = (m36, pr)

            # the normalize multiply is emitted separately so the DVE queue
            # can run row-half-0 products before row-half-1 norms
            norm_ops = {}

            def emit_norm(blk):
                y0 = 8 * blk
                m36, pr = norm_ops[blk]
                nc.vector.tensor_mul(mu16[:, y0 : y0 + 8, :], m36, pr)

            bounce = {}
            bc_first = [None]

            def emit_bounce(rh, h0=0, nh=16):
                dst = bass.AP(tensor=mu_dram,
                              offset=rh * 36 * 16 * W + h0 * W,
                              ap=[[16 * W, 36], [1, nh * W]])
                r0 = 16 * rh + h0
                bounce.setdefault(rh, []).append(
                    nc.sync.dma_start(out=dst, in_=mu16[:, r0 : r0 + nh, :]))

            # ---------------- reassembly chunk (rh, q, rows) -------------
            def emit_chunk(rh, q, h0=0, nh=16, split_bc=False, store=False):
                r1, r2 = q >> 1, q & 1
                mfull = mc.tile([128, 9, 16, W], f16, tag="mcast",
                                name=f"mc_{rh}_{q}_{h0}")
                mcast = mfull[:, :, 0:nh, :]
                mflat = mfull.rearrange("p t h w -> p (t h w)")
                base = (rh * 4 + q) * 9 * 16 * W
                if split_bc:
                    # one DMA per dx-group (still contiguous): products for
                    # group g start after only g+1 thirds of the broadcast
                    for g in range(3):
                        src = bass.AP(tensor=mu_dram,
                                      offset=base + g * 3 * 16 * W,
                                      ap=[[0, 128], [1, 3 * 16 * W]])
                        bc = nc.gpsimd.dma_start(
                            out=mflat[:, g * 3 * 16 * W : (g + 1) * 3 * 16 * W],
                            in_=src)
                        for bn in bounce[rh]:
                            add_dep_helper(bc.ins, bn.ins, sync=True,
                                           reason="mask broadcast after bounce")
                else:
                    src = bass.AP(tensor=mu_dram, offset=base,
                                  ap=[[0, 128], [1, 9 * 16 * W]])
                    bc = nc.gpsimd.dma_start(out=mflat, in_=src)
                    for bn in bounce[rh]:
                        add_dep_helper(bc.ins, bn.ins, sync=True,
                                       reason="mask broadcast after bounce")
                    if bc_first[0] is not None and (rh, q) == (0, 1):
                        add_dep_helper(bc.ins, bc_first[0].ins, sync=True,
                                       reason="first bc gets all engines")
                    if (rh, q) == (0, 0):
                        bc_first[0] = bc

                tfull = tp.tile([128, 9, 16, W], f16, tag="tmp",
                              name=f"tmp_{rh}_{q}_{h0}")
                tmp = tfull[:, :, 0:nh, :]

                def win(dxi, dy0, ndy):
                    r0 = 16 * rh + h0 + dy0
                    basep = x16[dxi][:, r0 : r0 + nh, :]
                    pdim = [list(p) for p in basep.ap][0]
                    return bass.AP(tensor=basep.tensor, offset=basep.offset,
                                   ap=[pdim, [W, ndy], [W, nh], [1, W]])

                # products: one DVE op per dx group (dy-triples share an
                # overlapping-row window AP); gpsimd is too slow and thrashes
                # the shared SBUF ports
                nc.vector.tensor_mul(tmp[:, 0:3], win(0, 0, 3), mcast[:, 0:3])
                nc.vector.tensor_mul(tmp[:, 3:6], win(1, 0, 3), mcast[:, 3:6])
                nc.vector.tensor_mul(tmp[:, 6:9], win(2, 0, 3), mcast[:, 6:9])

                tflat = tfull.rearrange("p t h w -> p t (h w)")
                for b in range(nh // 8):
                    acc = psa.tile([C, 8, W], f32, tag="acc",
                                   name=f"acc_{rh}_{q}_{h0}_{b}")
                    accf = acc.rearrange("p h w -> p (h w)")
                    for t in range(9):
                        nc.tensor.matmul(
                            accf, id_sb,
                            tflat[:, t, 512 * b : 512 * (b + 1)],
                            start=(t == 0), stop=(t == 8),
                            skip_group_check=True)
                    hb = h0 + 8 * b
                    nc.scalar.copy(
                        out16[rh][:, hb : hb + 8, r1, :, r2], acc)
                    if store and nh == 16:
                        # final chunk: ship each half-row-block the moment its
                        # last evacuation lands. Block 0 rides the idle sync
                        # queue so its issue cannot delay block 1's evac.
                        r0 = 32 * rh + 16 * b
                        eng = nc.sync if b == 0 else nc.scalar
                        eng.dma_start(out=out[:, r0 : r0 + 16, :],
                                      in_=out16[rh][:, 8 * b : 8 * b + 8])

            def emit_store(rh):
                for b in range(2):
                    r0 = 32 * rh + 16 * b
                    eng = nc.scalar if b == 0 else nc.sync
                    eng.dma_start(out=out[:, r0 : r0 + 16, :],
                                  in_=out16[rh][:, 8 * b : 8 * b + 8])

            # junk matmuls bridge PE dependency gaps so the HAM clock gate
            # never sees an idle window during the mask phase
            def warm_fill(n, pool=None):
                # reassembly-phase fills use the psc pool (idle by then) so
                # they never contend with the rotating acc tiles
                wt = (pool or psa).tile(
                    [68 if pool else 128, 8, W], f32,
                    tag="psc" if pool else "acc", name="wf")
                for _ in range(n):
                    nc.tensor.matmul(wt[:, 0:8, :], id_sb[:, 0:68] if pool
                                     else id_sb, blob[:, 0:512],
                                     start=True, stop=True,
                                     skip_group_check=True)

            # ---------------- emission schedule ----------------
            # PE queue runs every mask matmul before the reassembly
            # accumulations; DVE queue runs row-half-0 products before
            # row-half-1 norms, so neither engine stalls on the other.
            warm_fill(2)
            warm_fill(2)
            emit_mask_block(0)
            warm_fill(2)
            warm_fill(2)
            emit_mask_block(1)
            warm_fill(2)
            emit_norm(0)
            emit_bounce(0, 0, 8)
            emit_norm(1)
            emit_bounce(0, 8, 8)
            # row-half 1 mask matmuls fill PE while broadcasts stream
            emit_mask_block(2)
            warm_fill(2)
            emit_mask_block(3)
            warm_fill(2)
            warm_fill(20, pool=psc)
            emit_chunk(0, 0)
            emit_norm(2)
            emit_norm(3)
            emit_bounce(1)
            emit_chunk(0, 1)
            emit_chunk(0, 2)
            emit_chunk(0, 3)
            emit_store(0)
            emit_chunk(1, 0)
            emit_chunk(1, 1)
            emit_chunk(1, 2)
            emit_chunk(1, 3, store=True)

    nc.compile()
    return nc


def _get_program():
    global _PROGRAM
    if _PROGRAM is None:
        _PROGRAM = _build_program()
    return _PROGRAM


def _build_wblob(comp_w, comp_b, enc_w, enc_b):
    blob = np.zeros((C, NCOL), dtype=np.float16)
    blob[:, O_ID : O_ID + 128] = np.eye(128, dtype=np.float16)
    # fuse the 1x1 compress conv into the 3x3 encoder: exact because the
    # encoder's zero-pad commutes through a bias-free 1x1 conv (comp_b is
    # folded into the encoder bias separately)
    wfull = np.einsum("kcij,cd->kdij", enc_w.astype(np.float64),
                      comp_w[:, :, 0, 0].astype(np.float64))
    for di in range(3):
        for dj in range(3):
            t = di * 3 + dj
            blob[:, O_WF + KC * t : O_WF + KC * (t + 1)] = \
                wfull[:, :, di, dj].T.astype(np.float16)
    A40 = _build_A()
    blob[0:KC, O_A : O_A + 36] = A40[:, 0:36].astype(np.float16)
    blob[0:KC, O_A + 64 : O_A + 68] = A40[:, 36:40].astype(np.float16)
    R9 = np.zeros((4, 36), dtype=np.float16)
    for q in range(4):
        R9[q, q * 9 : (q + 1) * 9] = 1.0
    blob[0:4, O_R9 : O_R9 + 36] = R9
    # fold comp_b through the encoder taps into the encoder bias; bit-pack
    # the fp32 bias into two fp16 columns
    eb_eff = (enc_b.astype(np.float64)
              + enc_w.astype(np.float64).sum(axis=(2, 3))
              @ comp_b.astype(np.float64)).astype("<f4")
    blob[0:KC, O_EB : O_EB + 2] = eb_eff.reshape(KC, 1).view(np.float16)
    return blob


def _shard_inputs(x, comp_w, comp_b, enc_w, enc_b):
    wblob = np.ascontiguousarray(_build_wblob(comp_w, comp_b, enc_w, enc_b))
    in_maps = []
    for core in range(NCORES):
        b, h = divmod(core, 2)
        xs = np.zeros((C, HS + 2, W), dtype=np.float32)
        lo = h * HS - 1
        s0, s1 = max(0, lo), min(H, lo + HS + 2)
        xs[:, s0 - lo : s1 - lo, :] = x[b, :, s0:s1, :]
        in_maps.append({"xs": np.ascontiguousarray(xs), "wblob": wblob})
    return in_maps


def _run(inputs, trace=False):
    from concourse.bass_utils import run_bass_kernel_spmd

    nc = _get_program()
    in_maps = _shard_inputs(**inputs)
    res = run_bass_kernel_spmd(nc, in_maps, list(range(NCORES)), trace=trace)
    out = np.empty((B, C, 2 * H, 2 * W), dtype=np.float32)
    for core in range(NCORES):
        b, h = divmod(core, 2)
        out[b, :, h * 2 * HS : (h + 1) * 2 * HS, :] = \
            res.results[core]["out"].astype(np.float32)
    return out, res.exec_time_ns


def kernel(x, comp_w, comp_b, enc_w, enc_b):
    out, _ = _run(dict(x=np.asarray(x), comp_w=np.asarray(comp_w),
                       comp_b=np.asarray(comp_b), enc_w=np.asarray(enc_w),
                       enc_b=np.asarray(enc_b)))
    return out


# revision 59
# speedup vs baseline: 1.1328x; 1.1328x over previous
"""CARAFE content-aware upsampling (scale=2, K=5, encoder 3x3) on 8 TRN2 NeuronCores.

Sharding: 8 shards = batch(4) x H-halves(2), pure data parallel (1-row halo
per shard handled host-side). Channel-major fp16 pipeline, fully pipelined at
(row-half x subgrid) granularity:

  1+2. fused conv           : the 1x1 compress conv is folded into the 3x3
                              encoder host-side (W_enc @ W_comp, exact since
                              zero-pad commutes through a bias-free 1x1);
                              9 accumulating PE matmuls per 8-row block read
                              the dx-shifted x copies directly
  3. e = exp(enc + b)       : ACT (comp_b folded into enc bias host-side)
  4. combined masses        : one PE matmul [100->68] = 36 shifted-tap masses
                              (dx-major order, partitions 0-35) + 4 softmax
                              denominators S (partitions 64-67)
  5. r = 1/S                : DVE reciprocal_approx_fast (via SBUF staging);
                              R9 one-hot PE matmul expands r to 36 rows;
                              DVE multiplies masses in place
  6. mask broadcast         : per (row-half, subgrid): bounce to DRAM on the
                              sync HWDGE ring, then a stride-0 SWDGE DMA
                              (16 SDMA engines, ~400 GB/s aggregate)
                              replicating [9,16,64] masses to all 128
                              partitions
  7. reassembly             : 3 DVE multiplies per chunk (dy-triples share an
                              overlapping-row window AP); 9 accumulating PE
                              matmuls per 512-px block sum the taps in PSUM
  8. out = fp16 staging     : ACT PSUM->SBUF interleaved copy; two 0.5MB
                              store DMAs per row-half split across the two
                              HWDGE rings; host casts fp16 -> fp32

All weights ride in a single [128, 1134] fp16 blob (one DMA, loaded first);
the fp32 encoder bias is bit-packed into two fp16 columns. Junk matmuls
(initial burst + gap fillers) hold the PE HAM clock gate at 8/8 (2.4 GHz)
through the mask phase and into reassembly; the Exp ACT table is preloaded
off the critical path.
"""

import numpy as np

SCALE, KK, EK = 2, 5, 3
B, C, H, W = 4, 128, 64, 64
CC, KC = 64, 100
HS = H // 2          # 32 interior rows per shard
PIX = HS * W
NCORES = 8

# blob column offsets (fp16 columns)
O_ID, O_WF, O_A, O_R9, O_EB = 0, 128, 1028, 1096, 1132
NCOL = 1134

_PROGRAM = None


def _build_A():
    """[100, 40] combine matrix: cols 0-35 = shifted-tap masses (dx-major
    within each subgrid), cols 36-39 = softmax denominators per subgrid."""
    A = np.zeros((KC, 40), dtype=np.float32)
    for r1 in range(2):
        for r2 in range(2):
            q = 2 * r1 + r2
            for i in range(KK):
                for j in range(KK):
                    dy = (r1 + i - 2) // 2
                    dx = (r2 + j - 2) // 2
                    tidx = (dx + 1) * 3 + (dy + 1)
                    A[4 * (5 * i + j) + q, q * 9 + tidx] += 1.0
            A[np.arange(q, KC, 4), 36 + q] = 1.0
    return A


def _build_program():
    import concourse.bass as bass
    import concourse.tile as tile
    from concourse.tile import add_dep_helper
    from concourse import bacc, mybir

    f32 = mybir.dt.float32
    f16 = mybir.dt.float16
    AF = mybir.ActivationFunctionType

    nc = bacc.Bacc("TRN2", target_bir_lowering=False, debug=False,
                   num_devices=NCORES)

    xin = nc.declare_dram_parameter("xs", [C, HS + 2, W], f32, isOutput=False)
    wbl = nc.declare_dram_parameter("wblob", [C, NCOL], f16, isOutput=False)
    out = nc.declare_dram_parameter("out", [C, 2 * HS, 2 * W], f16, isOutput=True)

    # masks bounced per row-half: [rh][q][tap][16][64] fp16
    mu_dram = nc.dram_tensor("mu_bounce", [2, 4, 9, 16, W], f16)

    with tile.TileContext(nc) as tc:
        with (
            tc.tile_pool(name="singles", bufs=1) as singles,
            tc.tile_pool(name="work", bufs=2) as work,
            tc.tile_pool(name="mc", bufs=3) as mc,
            tc.tile_pool(name="tp", bufs=3) as tp,
            tc.tile_pool(name="pse", bufs=2, space="PSUM") as pse,
            tc.tile_pool(name="psc", bufs=1, space="PSUM") as psc,
            tc.tile_pool(name="psr", bufs=1, space="PSUM") as psr,
            tc.tile_pool(name="psa", bufs=4, space="PSUM") as psa,
        ):
            # ---------------- persistent SBUF ----------------
            blob = singles.tile([C, NCOL], f16, tag="wblob")
            x16 = [singles.tile([C, HS + 2, W], f16, tag=f"x16_{d}",
                                name=f"x16_{d}")
                   for d in range(3)]  # dx = -1, 0, +1 pre-shifted copies
            e_sb = singles.tile([KC, HS, W], f16, tag="e_sb")
            mu16 = singles.tile([36, HS, W], f16, tag="mu16")
            out16 = [singles.tile([C, 16, 2, W, 2], f16, tag=f"o16_{rh}",
                                  name=f"o16_{rh}")
                     for rh in range(2)]

            id_sb = blob[:, O_ID : O_ID + 128]
            wf_t = [blob[:, O_WF + KC * t : O_WF + KC * (t + 1)]
                    for t in range(9)]
            A_sb = blob[0:KC, O_A : O_A + 68]
            R9_sb = blob[0:4, O_R9 : O_R9 + 36]
            eb_sb = blob[0:KC, O_EB : O_EB + 2].bitcast(f32)

            # ---------------- loads ----------------
            nc.sync.dma_start(out=blob, in_=wbl[:])
            # x load with fp32 -> fp16 cast (SWDGE), 4 splits for fast ramp
            for a, b in [(0, 9), (9, 17), (17, 25), (25, 34)]:
                nc.gpsimd.dma_start(out=x16[1][:, a:b, :], in_=xin[:, a:b, :])

            # PE warm-up: back-to-back junk matmuls while the x-load streams,
            # so the HAM clock gate reaches 8/8 (2.4 GHz) before the real
            # mask matmuls start instead of running them all at 1.2 GHz.
            warm = psa.tile([128, 8, W], f32, tag="acc", name="warmup")
            for _ in range(40):
                nc.tensor.matmul(warm[:, 0:2, :], id_sb, id_sb,
                                 start=True, stop=True, skip_group_check=True)

            # preload the Exp ACT table during the load phase so the first
            # real exp doesn't pay the ~1.3us table load on the mask chain
            tblw = work.tile([1, 1], f16, tag="tblw")
            nc.scalar.activation(tblw, blob[0:1, 0:1], AF.Exp)

            nc.vector.memset(x16[0][:, :, 0:1], 0.0)
            nc.vector.memset(x16[2][:, :, W - 1 : W], 0.0)
            # dx-shift copies split per x-load chunk so encoder block 0 can
            # start right after the first chunk lands instead of after the
            # whole x-load
            for a, b in [(0, 9), (9, 17), (17, 25), (25, 34)]:
                nc.vector.tensor_copy(x16[0][:, a:b, 1:W],
                                      x16[1][:, a:b, 0 : W - 1])
                nc.vector.tensor_copy(x16[2][:, a:b, 0 : W - 1],
                                      x16[1][:, a:b, 1:W])

            # ---------------- stages 2-5 for one 8-row block ----------------
            def emit_mask_block(blk):
                y0 = 8 * blk
                ps = pse.tile([KC, 8, W], f32, tag="pse", name=f"enc_{y0}")
                # fused compress+encoder conv: 9 accumulating matmuls read
                # the dx-shifted x copies directly (zeroed edge columns
                # implement the conv padding exactly)
                for di in range(3):
                    for dj in range(3):
                        t = di * 3 + dj
                        nc.tensor.matmul(ps, wf_t[t],
                                         x16[dj][:, y0 + di : y0 + di + 8, :],
                                         start=(t == 0), stop=(t == 8))
                nc.scalar.activation(e_sb[:, y0 : y0 + 8, :], ps, AF.Exp,
                                     bias=eb_sb, scale=1.0)
                # combine: 36 masses + 4 denominators in one matmul
                pc = psc.tile([68, 8, W], f32, tag="psc", name=f"cmb_{y0}")
                nc.tensor.matmul(pc, A_sb, e_sb[:, y0 : y0 + 8, :],
                                 start=True, stop=True)
                s32 = work.tile([4, 8, W], f32, tag="s32", name=f"s32_{y0}")
                r32 = work.tile([4, 8, W], f32, tag="r32", name=f"r32_{y0}")
                r16 = work.tile([4, 8, W], f16, tag="r16", name=f"r16_{y0}")
                m36 = work.tile([36, 8, W], f16, tag="m36", name=f"m36_{y0}")
                # the bitwise-seed reciprocal cannot read PSUM; stage via ACT
                # (a DVE copy here would head-of-line-block the products
                # behind row-half-1's PE combines)
                nc.scalar.copy(s32, pc[64:68])
                nc.vector.reciprocal_approx_fast(r32, s32)
                nc.vector.tensor_copy(r16, r32)
                nc.scalar.copy(m36, pc[0:36])
                pr = psr.tile([36, 8, W], f32, tag="psr", name=f"r36_{y0}")
                nc.tensor.matmul(pr, R9_sb, r16, start=True, stop=True)
                norm_ops[blk] = (m36, pr)

            # the normalize multiply is emitted separately so the DVE queue
            # can run row-half-0 products before row-half-1 norms
            norm_ops = {}

            def emit_norm(blk):
                y0 = 8 * blk
                m36, pr = norm_ops[blk]
                nc.vector.tensor_mul(mu16[:, y0 : y0 + 8, :], m36, pr)

            bounce = {}
            bc_first = [None]

            def emit_bounce(rh, h0=0, nh=16):
                dst = bass.AP(tensor=mu_dram,
                              offset=rh * 36 * 16 * W + h0 * W,
                              ap=[[16 * W, 36], [1, nh * W]])
                r0 = 16 * rh + h0
                bounce.setdefault(rh, []).append(
                    nc.sync.dma_start(out=dst, in_=mu16[:, r0 : r0 + nh, :]))

            # ---------------- reassembly chunk (rh, q, rows) -------------
            def emit_chunk(rh, q, h0=0, nh=16, split_bc=False):
                r1, r2 = q >> 1, q & 1
                mfull = mc.tile([128, 9, 16, W], f16, tag="mcast",
                                name=f"mc_{rh}_{q}_{h0}")
                mcast = mfull[:, :, 0:nh, :]
                mflat = mfull.rearrange("p t h w -> p (t h w)")
                base = (rh * 4 + q) * 9 * 16 * W
                if split_bc:
                    # one DMA per dx-group (still contiguous): products for
                    # group g start after only g+1 thirds of the broadcast
                    for g in range(3):
                        src = bass.AP(tensor=mu_dram,
                                      offset=base + g * 3 * 16 * W,
                                      ap=[[0, 128], [1, 3 * 16 * W]])
                        bc = nc.gpsimd.dma_start(
                            out=mflat[:, g * 3 * 16 * W : (g + 1) * 3 * 16 * W],
                            in_=src)
                        for bn in bounce[rh]:
                            add_dep_helper(bc.ins, bn.ins, sync=True,
                                           reason="mask broadcast after bounce")
                else:
                    src = bass.AP(tensor=mu_dram, offset=base,
                                  ap=[[0, 128], [1, 9 * 16 * W]])
                    bc = nc.gpsimd.dma_start(out=mflat, in_=src)
                    for bn in bounce[rh]:
                        add_dep_helper(bc.ins, bn.ins, sync=True,
                                       reason="mask broadcast after bounce")
                    if bc_first[0] is not None and (rh, q) == (0, 1):
                        add_dep_helper(bc.ins, bc_first[0].ins, sync=True,
                                       reason="first bc gets all engines")
                    if (rh, q) == (0, 0):
                        bc_first[0] = bc

                tfull = tp.tile([128, 9, 16, W], f16, tag="tmp",
                              name=f"tmp_{rh}_{q}_{h0}")
                tmp = tfull[:, :, 0:nh, :]

                def win(dxi, dy0, ndy):
                    r0 = 16 * rh + h0 + dy0
                    basep = x16[dxi][:, r0 : r0 + nh, :]
                    pdim = [list(p) for p in basep.ap][0]
                    return bass.AP(tensor=basep.tensor, offset=basep.offset,
                                   ap=[pdim, [W, ndy], [W, nh], [1, W]])

                # products: one DVE op per dx group (dy-triples share an
                # overlapping-row window AP); gpsimd is too slow and thrashes
                # the shared SBUF ports
                nc.vector.tensor_mul(tmp[:, 0:3], win(0, 0, 3), mcast[:, 0:3])
                nc.vector.tensor_mul(tmp[:, 3:6], win(1, 0, 3), mcast[:, 3:6])
                nc.vector.tensor_mul(tmp[:, 6:9], win(2, 0, 3), mcast[:, 6:9])

                tflat = tfull.rearrange("p t h w -> p t (h w)")
                for b in range(nh // 8):
                    acc = psa.tile([C, 8, W], f32, tag="acc",
                                   name=f"acc_{rh}_{q}_{h0}_{b}")
                    accf = acc.rearrange("p h w -> p (h w)")
                    for t in range(9):
                        nc.tensor.matmul(
                            accf, id_sb,
                            tflat[:, t, 512 * b : 512 * (b + 1)],
                            start=(t == 0), stop=(t == 8),
                            skip_group_check=True)
                    hb = h0 + 8 * b
                    nc.scalar.copy(
                        out16[rh][:, hb : hb + 8, r1, :, r2], acc)


            def emit_store(rh):
                for b in range(2):
                    r0 = 32 * rh + 16 * b
                    eng = nc.scalar if b == 0 else nc.sync
                    eng.dma_start(out=out[:, r0 : r0 + 16, :],
                                  in_=out16[rh][:, 8 * b : 8 * b + 8])

            # junk matmuls bridge PE dependency gaps so the HAM clock gate
            # never sees an idle window during the mask phase
            def warm_fill(n, pool=None):
                # reassembly-phase fills use the psc pool (idle by then) so
                # they never contend with the rotating acc tiles
                wt = (pool or psa).tile(
                    [68 if pool else 128, 8, W], f32,
                    tag="psc" if pool else "acc", name="wf")
                for _ in range(n):
                    nc.tensor.matmul(wt[:, 0:8, :], id_sb[:, 0:68] if pool
                                     else id_sb, blob[:, 0:512],
                                     start=True, stop=True,
                                     skip_group_check=True)

            # ---------------- emission schedule ----------------
            # PE queue runs every mask matmul before the reassembly
            # accumulations; DVE queue runs row-half-0 products before
            # row-half-1 norms, so neither engine stalls on the other.
            warm_fill(2)
            warm_fill(2)
            emit_mask_block(0)
            warm_fill(2)
            warm_fill(2)
            emit_mask_block(1)
            warm_fill(2)
            emit_norm(0)
            emit_bounce(0, 0, 8)
            emit_norm(1)
            emit_bounce(0, 8, 8)
            # row-half 1 mask matmuls fill PE while broadcasts stream
            emit_mask_block(2)
            warm_fill(2)
            emit_mask_block(3)
            warm_fill(2)
            warm_fill(20, pool=psc)
            emit_chunk(0, 0)
            emit_norm(2)
            emit_norm(3)
            emit_bounce(1)
            emit_chunk(0, 1)
            emit_chunk(0, 2)
            emit_chunk(0, 3)
            emit_store(0)
            emit_chunk(1, 0)
            emit_chunk(1, 1)
            emit_chunk(1, 2)
            emit_chunk(1, 3)
            emit_store(1)

    nc.compile()
    return nc


def _get_program():
    global _PROGRAM
    if _PROGRAM is None:
        _PROGRAM = _build_program()
    return _PROGRAM


def _build_wblob(comp_w, comp_b, enc_w, enc_b):
    blob = np.zeros((C, NCOL), dtype=np.float16)
    blob[:, O_ID : O_ID + 128] = np.eye(128, dtype=np.float16)
    # fuse the 1x1 compress conv into the 3x3 encoder: exact because the
    # encoder's zero-pad commutes through a bias-free 1x1 conv (comp_b is
    # folded into the encoder bias separately)
    wfull = np.einsum("kcij,cd->kdij", enc_w.astype(np.float64),
                      comp_w[:, :, 0, 0].astype(np.float64))
    for di in range(3):
        for dj in range(3):
            t = di * 3 + dj
            blob[:, O_WF + KC * t : O_WF + KC * (t + 1)] = \
                wfull[:, :, di, dj].T.astype(np.float16)
    A40 = _build_A()
    blob[0:KC, O_A : O_A + 36] = A40[:, 0:36].astype(np.float16)
    blob[0:KC, O_A + 64 : O_A + 68] = A40[:, 36:40].astype(np.float16)
    R9 = np.zeros((4, 36), dtype=np.float16)
    for q in range(4):
        R9[q, q * 9 : (q + 1) * 9] = 1.0
    blob[0:4, O_R9 : O_R9 + 36] = R9
    # fold comp_b through the encoder taps into the encoder bias; bit-pack
    # the fp32 bias into two fp16 columns
    eb_eff = (enc_b.astype(np.float64)
              + enc_w.astype(np.float64).sum(axis=(2, 3))
              @ comp_b.astype(np.float64)).astype("<f4")
    blob[0:KC, O_EB : O_EB + 2] = eb_eff.reshape(KC, 1).view(np.float16)
    return blob


def _shard_inputs(x, comp_w, comp_b, enc_w, enc_b):
    wblob = np.ascontiguousarray(_build_wblob(comp_w, comp_b, enc_w, enc_b))
    in_maps = []
    for core in range(NCORES):
        b, h = divmod(core, 2)
        xs = np.zeros((C, HS + 2, W), dtype=np.float32)
        lo = h * HS - 1
        s0, s1 = max(0, lo), min(H, lo + HS + 2)
        xs[:, s0 - lo : s1 - lo, :] = x[b, :, s0:s1, :]
        in_maps.append({"xs": np.ascontiguousarray(xs), "wblob": wblob})
    return in_maps


def _run(inputs, trace=False):
    from concourse.bass_utils import run_bass_kernel_spmd

    nc = _get_program()
    in_maps = _shard_inputs(**inputs)
    res = run_bass_kernel_spmd(nc, in_maps, list(range(NCORES)), trace=trace)
    out = np.empty((B, C, 2 * H, 2 * W), dtype=np.float32)
    for core in range(NCORES):
        b, h = divmod(core, 2)
        out[b, :, h * 2 * HS : (h + 1) * 2 * HS, :] = \
            res.results[core]["out"].astype(np.float32)
    return out, res.exec_time_ns


def kernel(x, comp_w, comp_b, enc_w, enc_b):
    out, _ = _run(dict(x=np.asarray(x), comp_w=np.asarray(comp_w),
                       comp_b=np.asarray(comp_b), enc_w=np.asarray(enc_w),
                       enc_b=np.asarray(enc_b)))
    return out
